# revision 50
# baseline (speedup 1.0000x reference)
"""DigitCaps dynamic-routing kernel for 8 trn2 NeuronCores.

Math (reference):
    u_hat[b,r,c,o] = sum_i W[r,c,o,i] * x[b,r,i]        # never materialized!
    3 routing iters:
        c_ij = softmax(b_ij, axis=r)                     # (R, C)
        s    = einsum('rc,brco->bco', c_ij, u_hat)
        v    = squash(s)  elementwise: s|s|/(1+s^2)
        b_ij += mean_b <u_hat[b,r,c,:], v[b,c,:]>

Key restructuring:
  * s[b,(c,o)] = sum_{k=(r,i)} X[b,k] * (exp(b_ij[r,c]) * Wp[k,(c,o)]) / D[c]
    with Wp[(r,i),(c,o)] = W[r,c,o,i] and D[c] = sum_r exp(b_ij[r,c]).
    The softmax normalizer D commutes through the contraction, so each
    routing iteration needs exactly ONE AllReduce (partial s-tilde + partial D).
  * agreement: m[r,c] = (1/B) sum_{i,o} Wp[(r,i),(c,o)] * G[(r,i),(c,o)]
    with G = X^T V  (contraction over batch) -- a second dense matmul.
  * Routes are sharded 8-way (144 routes / core, K=1152 contraction rows);
    batch is replicated. The only cross-core traffic is the per-iteration
    AllReduce of (256x160 partial s + 10 partial D) = ~160KB.
  * The final iteration's AllReduce is folded into the host-side gather:
    each core emits its raw partial s-tilde and partial D; the host sums,
    normalizes and applies the squash.

Precision: plain fp32 matmuls (4 cyc/row on the PE). _USE_F32R=True switches
the routing-iteration matmuls to float32r (full-rate streaming with the
moving dim padded to 256, ~8us faster) but hardware f32r rounding costs
~1.2e-4 relative error through the routing-weight path vs 2.8e-6 at fp32,
which risks a scale-relative + fp32-envelope absmax gate — so it ships off.
"""

import numpy as np

import concourse.bass as bass
import concourse.mybir as mybir
from concourse import bacc, tile
from concourse.bass_utils import run_bass_kernel_spmd

B, R, C, O, I = 256, 1152, 10, 16, 8
NCORES = 8
RL = R // NCORES            # 144 routes per core
KL = RL * I                 # 1152 contraction rows per core
KT = KL // 128              # 9 K-tiles of 128
CO = C * O                  # 160
MB = B // 128               # 2 batch blocks of 128
NITER = 3

_USE_F32R = False
NPAD = 256 if _USE_F32R else CO   # moving-dim padding for full-rate f32r

F32 = mybir.dt.float32
F32R = mybir.dt.float32r
MMDT = F32R if _USE_F32R else F32
AF = mybir.ActivationFunctionType
ALU = mybir.AluOpType

_CACHE = {}


def _f32(ap):
    """View a (possibly float32r) AP as plain float32 for DVE/ACT ops."""
    return ap.bitcast(F32) if ap.dtype != F32 else ap


def _free_bcast(ap, dims, extra_offset=0):
    """Manual AP keeping the partition dim with custom free dims
    (steps in elements, step 0 = broadcast)."""
    return bass.AP(
        ap.tensor, ap.offset + extra_offset,
        [list(ap.ap[0])] + [list(d) for d in dims],
    )


def _build_nc(variant="full"):
    nc = bacc.Bacc(
        trn_type="TRN2",
        target_bir_lowering=False,
        debug=False,
        num_devices=NCORES,
    )

    xt_d = nc.dram_tensor("xt", [KL, B], MMDT, kind="ExternalInput")      # (r,i) x b
    xn_d = nc.dram_tensor("xn", [B, KL], F32, kind="ExternalInput")       # b x (r,i)
    wt_d = nc.dram_tensor("wt", [KL, NPAD], MMDT, kind="ExternalInput")   # padded Wp
    jm_d = nc.dram_tensor("jm", [128, 128], F32, kind="ExternalInput")    # kron(I16,1_8)/B
    oc_d = nc.dram_tensor("oc", [128, 1], F32, kind="ExternalInput")      # 1/I column
    outs_d = nc.dram_tensor("out_s", [MB, 128, CO], F32, kind="ExternalOutput")
    outd_d = nc.dram_tensor("out_d", [C, 1], F32, kind="ExternalOutput")

    with tile.TileContext(nc) as tc:
        _body(tc, xt_d, xn_d, wt_d, jm_d, oc_d, outs_d, outd_d, variant)
    nc.compile()
    return nc


def _body(tc, xt_d, xn_d, wt_d, jm_d, oc_d, outs_d, outd_d, variant="full"):
    nc = tc.nc
    ts = bass.ts

    with (
        tc.tile_pool(name="sb", bufs=1) as sb,
        tc.tile_pool(name="pss", bufs=1, space="PSUM") as pss,
        tc.tile_pool(name="psg", bufs=3, space="PSUM") as psg,
        tc.tile_pool(name="psx", bufs=1, space="PSUM") as psx,
        tc.tile_pool(name="psj", bufs=1, space="PSUM") as psj,
        tc.tile_pool(name="dram", bufs=1, space="DRAM") as dram,
    ):
        # ---- persistent SBUF tensors ----
        xt_s = sb.tile([128, KT * B], MMDT)       # lhsT for s-matmul
        xn_s = sb.tile([128, MB * KL], F32)      # lhsT for G-matmul
        wt_s = sb.tile([128, KT * NPAD], MMDT)    # Wp (padded)
        wp_s = sb.tile([128, KT * NPAD], MMDT)    # exp(b)-scaled Wp
        wpf_s = sb.tile([128, KT * CO], F32)      # fp32 W' for the final iter
        jm_s = sb.tile([128, 128], F32)
        oc_s = sb.tile([128, 1], F32)
        vv_s = sb.tile([128, MB * NPAD], F32)    # squashed v (padded)
        s_s = sb.tile([128, MB * CO], F32)        # summed s-tilde
        b_s = sb.tile([128, KT * C], F32)         # b_ij expanded over i
        # 6 tail cols so the padded W'-scale read (c-count 16 per tile,
        # zero-multiplied) stays in bounds
        ct_s = sb.tile([128, KT * C + 6], F32)    # exp(b_ij)
        cs_s = sb.tile([128, C], F32)             # sum_t exp(b)
        p_s = sb.tile([128, KT * CO], F32)        # Wp .* G scratch
        mio_s = sb.tile([128, KT * C], F32)       # sum_o (Wp .* G)
        a2_s = sb.tile([128, MB * CO], F32)       # s^2
        t2_s = sb.tile([128, MB * CO], F32)       # s^2 + D^2
        r2_s = sb.tile([128, MB * CO], F32)       # 1/(s^2+D^2)
        ab_s = sb.tile([128, MB * CO], F32)       # |s|
        nn_s = sb.tile([128, MB * CO], F32)       # s*|s|
        dbc_s = sb.tile([128, C], F32)            # D broadcast over partitions
        dsq_s = sb.tile([128, C], F32)            # D^2
        ev_s = sb.tile([128, MB * CO], F32)       # PSUM->SBUF s-tilde evac
        dv_s = sb.tile([C, 1], F32)               # PSUM->SBUF D evac
        jnk_s = sb.tile([128, NPAD], F32)         # junk src for PE warmup

        # ---- DRAM bounce buffers for the collectives ----
        cc0_in = dram.tile([MB, 128, CO], F32)
        cc0_out = dram.tile([MB, 128, CO], F32)
        CC1N = MB * 128 * CO + C
        cc1_in = dram.tile([CC1N], F32)
        cc1_out = dram.tile([CC1N], F32)

        # ---- PE warmup: the HAM clock-gate starts cold (1.2GHz). ~3.5us of
        # junk matmuls during the input-load window flips it to 2.4GHz before
        # the first real matmul group (which otherwise runs its first ~3.4us
        # at half rate). Sized to end as the first load chunks land. ----
        nc.vector.memset(jnk_s[:], 0.0)
        if variant == "full":
            junk0 = psj.tile([128, NPAD], F32, name="junk", tag="junk")
            for _ in range(10):
                nc.tensor.matmul(
                    junk0[:], lhsT=jnk_s[:, 0:128], rhs=jnk_s[:],
                    start=True, stop=True,
                )

        # ---- loads (xt+wt first, in K-tile chunks: iteration-0 matmuls
        # start as soon as the first chunk of both lands) ----
        xt_ld = xt_s.rearrange("p (t b) -> p t b", t=KT)
        xt_src = xt_d.ap().rearrange("(t p) b -> p t b", p=128)
        wt_ld = wt_s.rearrange("p (t f) -> p t f", t=KT)
        wt_src = wt_d.ap().rearrange("(t p) f -> p t f", p=128)
        for lo, hi in ((0, 3), (3, 6), (6, KT)):
            nc.sync.dma_start(out=xt_ld[:, lo:hi], in_=xt_src[:, lo:hi])
            nc.sync.dma_start(out=wt_ld[:, lo:hi], in_=wt_src[:, lo:hi])
        nc.sync.dma_start(out=jm_s[:], in_=jm_d.ap())
        nc.sync.dma_start(out=oc_s[:], in_=oc_d.ap())
        nc.sync.dma_start(
            out=xn_s.rearrange("p (m k) -> p m k", m=MB),
            in_=xn_d.ap().rearrange("(m p) k -> p m k", p=128),
        )

        # vv pad columns [CO:NPAD) of each m-block: filled once with zeros
        # DMA'd from wt's (host-zeroed) pad region — Memset can't emit f32r.
        nc.vector.memset(b_s[:], 0.0)
        nc.vector.memset(ct_s[:, KT * C:], 0.0)

        xt3 = xt_s.rearrange("p (t b) -> p t b", t=KT)
        xn3 = xn_s.rearrange("p (m k) -> p m k", m=MB)
        wt3 = wt_s.rearrange("p (t f) -> p t f", t=KT)
        wp3 = wp_s.rearrange("p (t f) -> p t f", t=KT)
        vv3 = vv_s.rearrange("p (m f) -> p m f", m=MB)
        s3 = s_s.rearrange("p (m f) -> p m f", m=MB)
        ev3 = ev_s.rearrange("p (m f) -> p m f", m=MB)

        def stilde_matmul(w3, emit_block, f32_exact=False):
            """18 accumulating matmuls, two PSUM tiles (separate banks).

            After each m-block's 9 matmuls finish, the block is evacuated to
            ev_s and handed to emit_block(m) (the DMA toward the collective
            buffer / output) — so block 0's DMA latency hides behind block 1's
            matmuls instead of trailing the whole group.

            f32_exact: full-precision fp32 matmuls (final iteration — its
            result IS the output; routing-weight iterations tolerate f32r)."""
            for m in range(MB):
                sp = pss.tile([128, NPAD], F32, name=f"sps{m}", tag=f"sps{m}")
                for t in range(KT):
                    if f32_exact:
                        nc.tensor.matmul(
                            sp[:, 0:CO],
                            lhsT=_f32(xt3[:, t, ts(m, 128)]),
                            rhs=w3[:, t, :],
                            start=(t == 0),
                            stop=(t == KT - 1),
                        )
                    else:
                        nc.tensor.matmul(
                            sp[:],
                            lhsT=xt3[:, t, ts(m, 128)],
                            rhs=w3[:, t, :],
                            start=(t == 0),
                            stop=(t == KT - 1),
                        )
                nc.scalar.activation(ev3[:, m, :], sp[:, 0:CO], AF.Copy)
                emit_block(m)

        def squash(d_const=None):
            """vv <- squash(s / D): v = s*|s| / (D^2 + s^2), all on DVE.

            Per m-block so the first G matmul can start after block 0."""
            if d_const is None:
                nc.vector.tensor_mul(dsq_s[:], dbc_s[:], dbc_s[:])
            for m in range(MB):
                sf = s3[:, m, :]
                a2 = a2_s.rearrange("p (m f) -> p m f", m=MB)[:, m, :]
                ab = ab_s.rearrange("p (m f) -> p m f", m=MB)[:, m, :]
                t2 = t2_s.rearrange("p (m f) -> p m f", m=MB)[:, m, :]
                r2 = r2_s.rearrange("p (m f) -> p m f", m=MB)[:, m, :]
                nn = nn_s.rearrange("p (m f) -> p m f", m=MB)[:, m, :]
                nc.vector.tensor_mul(a2, sf, sf)
                nc.vector.scalar_tensor_tensor(
                    ab, sf, -1.0, sf, op0=ALU.mult, op1=ALU.max)
                if d_const is not None:
                    nc.vector.tensor_scalar_add(t2, a2, float(d_const) ** 2)
                else:
                    nc.vector.tensor_add(
                        _free_bcast(t2_s, [[O, C], [1, O]],
                                    extra_offset=m * CO),
                        _free_bcast(a2_s, [[O, C], [1, O]],
                                    extra_offset=m * CO),
                        _free_bcast(dsq_s, [[1, C], [0, O]]),
                    )
                nc.vector.reciprocal(r2, t2)
                nc.vector.tensor_mul(nn, sf, ab)
                nc.vector.tensor_mul(vv3[:, m, 0:CO], nn, r2)

        def agreement(last=False):
            """G = X^T V; m = (1/B) sum_io Wp.*G; b += m; ct = exp(b); cs, D.

            last=True writes the scaled weights into the fp32 wpf_s tile for
            the exact final matmul instead of the f32r wp_s."""
            for t in range(KT):
                g_ps = psg.tile([128, NPAD], F32, name="gps", tag="gps")
                for m in range(MB):
                    nc.tensor.matmul(
                        g_ps[:, 0:CO],
                        lhsT=xn3[:, m, ts(t, 128)],
                        rhs=vv3[:, m, 0:CO],
                        start=(m == 0),
                        stop=(m == MB - 1),
                    )
                nc.vector.tensor_mul(
                    p_s[:, ts(t, CO)], _f32(wt3[:, t, 0:CO]), g_ps[:, 0:CO]
                )
                nc.vector.reduce_sum(
                    mio_s[:, ts(t, C)],
                    _free_bcast(p_s, [[16, C], [1, O]], extra_offset=t * CO),
                    axis=mybir.AxisListType.X,
                )
            me_ps = psx.tile([128, KT * C], F32, name="meps", tag="meps")
            nc.tensor.matmul(
                me_ps[:], lhsT=jm_s[:], rhs=mio_s[:], start=True, stop=True
            )
            nc.vector.tensor_add(b_s[:], b_s[:], me_ps[:])
            nc.scalar.activation(ct_s[:, 0:KT * C], b_s[:], AF.Exp)
            # W' = Wp * exp(b) per K-tile so the next matmul group pipelines
            # (broadcast exp(b) over o). The c-range runs to 16, not C=10,
            # covering the pad cols too: wt pads are 0, so wp pads = 0 with
            # no separate (f32r-incapable) memset.
            if last:
                for t in range(KT):
                    nc.vector.tensor_mul(
                        _free_bcast(wpf_s, [[16, C], [1, O]],
                                    extra_offset=t * CO),
                        _free_bcast(_f32(wt_s), [[16, C], [1, O]],
                                    extra_offset=t * NPAD),
                        _free_bcast(ct_s, [[1, C], [0, O]],
                                    extra_offset=t * C),
                    )
            else:
                cpt = NPAD // O
                for t in range(KT):
                    nc.vector.tensor_mul(
                        _free_bcast(wp_s, [[16, cpt], [1, O]],
                                    extra_offset=t * NPAD),
                        _free_bcast(_f32(wt_s), [[16, cpt], [1, O]],
                                    extra_offset=t * NPAD),
                        _free_bcast(ct_s, [[1, cpt], [0, O]],
                                    extra_offset=t * C),
                    )
            # D-tail after the W' loop: cs/D only ride the collective buffer
            # or output DMA, while W' tiles gate the next matmul group — so
            # let the DVE service W' first.
            nc.vector.reduce_sum(
                cs_s[:],
                _free_bcast(ct_s, [[1, C], [C, KT]]),
                axis=mybir.AxisListType.X,
            )
            d_ps = psx.tile([C, 1], F32, name="dps", tag="dps")
            nc.tensor.matmul(d_ps[:], lhsT=cs_s[:], rhs=oc_s[:], start=True, stop=True)
            nc.scalar.activation(dv_s[:], d_ps[:], AF.Copy)

        def allreduce(cc_in, cc_out):
            if variant == "nocc":
                nc.sync.dma_start(out=cc_out, in_=cc_in)
            else:
                nc.gpsimd.collective_compute(
                    "AllReduce",
                    mybir.AluOpType.add,
                    replica_groups=[list(range(NCORES))],
                    ins=[cc_in.opt()],
                    outs=[cc_out.opt()],
                )
                # PE stays idle ~17us through the AllReduce sandwich — past
                # the ~3.4us HAM window, so the post-sync matmul block would
                # restart at the cold 1.2GHz clock (~2x slower). Keep the HAM
                # warm with back-to-back junk matmuls sized to end ~3us
                # before the expected sync completion (PE tolerates <3.4us
                # of trailing idle without rethrottling). No data deps: they
                # fill the PE queue during the collective only.
                junk = psj.tile([128, NPAD], F32, name="junk", tag="junk")
                for _ in range(45):
                    nc.tensor.matmul(
                        junk[:], lhsT=xt3[:, 0, 0:128], rhs=wt3[:, 0, :],
                        start=True, stop=True,
                    )

        # ================= iteration 0 =================
        if variant == "mm_only":
            stilde_matmul(
                wt3, lambda m: nc.sync.dma_start(
                    out=outs_d.ap()[m], in_=ev3[:, m, :]))
            nc.sync.dma_start(out=outd_d.ap(), in_=ev_s[0:C, 0:1])
            return
        stilde_matmul(
            wt3, lambda m: nc.sync.dma_start(
                out=cc0_in[m], in_=ev3[:, m, :]))
        allreduce(cc0_in, cc0_out)
        for m in range(MB):
            nc.sync.dma_start(out=s3[:, m, :], in_=cc0_out[m])
        squash(d_const=float(R))           # D0 = R (softmax of zeros)
        agreement()                        # -> b1, exp(b1), D1 partial

        # ================= iteration 1 =================
        cc1s = cc1_in[0:MB * 128 * CO].rearrange("(m p f) -> m p f", m=MB, p=128)
        stilde_matmul(
            wp3, lambda m: nc.sync.dma_start(
                out=cc1s[m], in_=ev3[:, m, :]))
        nc.sync.dma_start(out=cc1_in[MB * 128 * CO:CC1N], in_=dv_s[:])
        allreduce(cc1_in, cc1_out)
        cc1o = cc1_out[0:MB * 128 * CO].rearrange("(m p f) -> m p f", m=MB, p=128)
        # order: s-block0, D, s-block1 — HWDGE serializes ~0.6us per DMA, and
        # squash block 0 needs D (via dsq) by its third op
        nc.sync.dma_start(out=s3[:, 0, :], in_=cc1o[0])
        # D row from DRAM, replicated across all 128 partitions
        nc.sync.dma_start(
            out=dbc_s[:],
            in_=bass.AP(cc1_out.tensor, cc1_out.offset + MB * 128 * CO,
                        [[0, 128], [1, C]]),
        )
        nc.sync.dma_start(out=s3[:, 1, :], in_=cc1o[1])
        squash()
        agreement(last=True)               # -> b2, exp(b2), D2 partial (fp32 W')

        # ================= iteration 2 (exact fp32: result is the output) ==
        stilde_matmul(
            wpf_s.rearrange("p (t f) -> p t f", t=KT),
            lambda m: nc.sync.dma_start(out=outs_d.ap()[m], in_=ev3[:, m, :]),
            f32_exact=True)
        nc.sync.dma_start(out=outd_d.ap(), in_=dv_s[:])


def _prep_inputs(x, W):
    x = np.ascontiguousarray(np.asarray(x, np.float32))
    W = np.asarray(W, np.float32)
    Wp = np.ascontiguousarray(W.transpose(0, 3, 1, 2).reshape(R * I, C * O))
    Wpad = np.zeros((R * I, NPAD), np.float32)
    Wpad[:, :CO] = Wp
    jm = (np.kron(np.eye(16, dtype=np.float32), np.ones((8, 8), np.float32)) / B
          ).astype(np.float32)
    oc = np.full((128, 1), 1.0 / I, np.float32)
    in_maps = []
    for k in range(NCORES):
        xs = x[:, k * RL:(k + 1) * RL, :].reshape(B, KL)
        in_maps.append({
            "xt": np.ascontiguousarray(xs.T),
            "xn": np.ascontiguousarray(xs),
            "wt": np.ascontiguousarray(Wpad[k * KL:(k + 1) * KL]),
            "jm": jm,
            "oc": oc,
        })
    return in_maps


def _postprocess(results):
    s = np.zeros((MB, 128, CO), np.float64)
    D = np.zeros((C, 1), np.float64)
    for r in results:
        s += r["out_s"].astype(np.float64)
        D += r["out_d"].astype(np.float64)
    s = s.reshape(B, C, O)
    sn = s / D.reshape(C)[None, :, None]
    v = sn * np.abs(sn) / (1.0 + sn * sn)
    return v[..., None].astype(np.float32)


def _get_nc(variant="full"):
    key = f"nc_{variant}"
    if key not in _CACHE:
        _CACHE[key] = _build_nc(variant)
    return _CACHE[key]


def run_on_hw(x, W, **kw):
    """Run the bass kernel on the 8 cores; kw forwarded (e.g. trace=True)."""
    nc = _get_nc()
    in_maps = _prep_inputs(x, W)
    res = run_bass_kernel_spmd(nc, in_maps, core_ids=list(range(NCORES)), **kw)
    return _postprocess(res.results), res


def kernel(x, W):
    out, _ = run_on_hw(x, W)
    return out


# revision 54
# speedup vs baseline: 1.2953x; 1.2953x over previous
"""DigitCaps dynamic-routing kernel for 8 trn2 NeuronCores.

Math (reference):
    u_hat[b,r,c,o] = sum_i W[r,c,o,i] * x[b,r,i]        # never materialized!
    3 routing iters:
        c_ij = softmax(b_ij, axis=r)                     # (R, C)
        s    = einsum('rc,brco->bco', c_ij, u_hat)
        v    = squash(s)  elementwise: s|s|/(1+s^2)
        b_ij += mean_b <u_hat[b,r,c,:], v[b,c,:]>

Key restructuring:
  * s[b,(c,o)] = sum_{k=(r,i)} X[b,k] * (exp(b_ij[r,c]) * Wp[k,(c,o)]) / D[c]
    with Wp[(r,i),(c,o)] = W[r,c,o,i] and D[c] = sum_r exp(b_ij[r,c]).
    The softmax normalizer D commutes through the contraction, so each
    routing iteration needs exactly ONE AllReduce (partial s-tilde + partial D).
  * agreement: m[r,c] = (1/B) sum_{i,o} Wp[(r,i),(c,o)] * G[(r,i),(c,o)]
    with G = X^T V  (contraction over batch) -- a second dense matmul.
  * Routes are sharded 8-way (144 routes / core, K=1152 contraction rows);
    batch is replicated. The only cross-core traffic is the per-iteration
    AllReduce of (256x160 partial s + 10 partial D) = ~160KB.
  * The final iteration's AllReduce is folded into the host-side gather:
    each core emits its raw partial s-tilde and partial D; the host sums,
    normalizes and applies the squash.

Precision: plain fp32 matmuls (4 cyc/row on the PE). _USE_F32R=True switches
the routing-iteration matmuls to float32r (full-rate streaming with the
moving dim padded to 256, ~8us faster) but hardware f32r rounding costs
~1.2e-4 relative error through the routing-weight path vs 2.8e-6 at fp32,
which risks a scale-relative + fp32-envelope absmax gate — so it ships off.
"""

import numpy as np

import concourse.bass as bass
import concourse.mybir as mybir
from concourse import bacc, tile
from concourse.bass_utils import run_bass_kernel_spmd

B, R, C, O, I = 256, 1152, 10, 16, 8
NCORES = 8
RL = R // NCORES            # 144 routes per core
KL = RL * I                 # 1152 contraction rows per core
KT = KL // 128              # 9 K-tiles of 128
CO = C * O                  # 160
MB = B // 128               # 2 batch blocks of 128
NITER = 3

_USE_F32R = False
NPAD = 256 if _USE_F32R else CO   # moving-dim padding for full-rate f32r

F32 = mybir.dt.float32
F32R = mybir.dt.float32r
MMDT = F32R if _USE_F32R else F32
AF = mybir.ActivationFunctionType
ALU = mybir.AluOpType

_CACHE = {}


def _f32(ap):
    """View a (possibly float32r) AP as plain float32 for DVE/ACT ops."""
    return ap.bitcast(F32) if ap.dtype != F32 else ap


def _free_bcast(ap, dims, extra_offset=0):
    """Manual AP keeping the partition dim with custom free dims
    (steps in elements, step 0 = broadcast)."""
    return bass.AP(
        ap.tensor, ap.offset + extra_offset,
        [list(ap.ap[0])] + [list(d) for d in dims],
    )


def _build_nc(variant="full"):
    nc = bacc.Bacc(
        trn_type="TRN2",
        target_bir_lowering=False,
        debug=False,
        num_devices=NCORES,
    )

    xt_d = nc.dram_tensor("xt", [KL, B], MMDT, kind="ExternalInput")      # (r,i) x b
    xn_d = nc.dram_tensor("xn", [B, KL], F32, kind="ExternalInput")       # b x (r,i)
    wt_d = nc.dram_tensor("wt", [KL, NPAD], MMDT, kind="ExternalInput")   # padded Wp
    jm_d = nc.dram_tensor("jm", [128, 128], F32, kind="ExternalInput")    # kron(I16,1_8)/B
    oc_d = nc.dram_tensor("oc", [128, 1], F32, kind="ExternalInput")      # 1/I column
    outs_d = nc.dram_tensor("out_s", [MB, 128, CO], F32, kind="ExternalOutput")
    outd_d = nc.dram_tensor("out_d", [C, 1], F32, kind="ExternalOutput")

    with tile.TileContext(nc) as tc:
        _body(tc, xt_d, xn_d, wt_d, jm_d, oc_d, outs_d, outd_d, variant)
    nc.compile()
    return nc


def _body(tc, xt_d, xn_d, wt_d, jm_d, oc_d, outs_d, outd_d, variant="full"):
    nc = tc.nc
    ts = bass.ts

    with (
        tc.tile_pool(name="sb", bufs=1) as sb,
        tc.tile_pool(name="pss", bufs=1, space="PSUM") as pss,
        tc.tile_pool(name="psg", bufs=3, space="PSUM") as psg,
        tc.tile_pool(name="psx", bufs=1, space="PSUM") as psx,
        tc.tile_pool(name="psj", bufs=1, space="PSUM") as psj,
        tc.tile_pool(name="dram", bufs=1, space="DRAM") as dram,
    ):
        # ---- persistent SBUF tensors ----
        xt_s = sb.tile([128, KT * B], MMDT)       # lhsT for s-matmul
        xn_s = sb.tile([128, MB * KL], F32)      # lhsT for G-matmul
        wt_s = sb.tile([128, KT * NPAD], MMDT)    # Wp (padded)
        wp_s = sb.tile([128, KT * NPAD], MMDT)    # exp(b)-scaled Wp
        wpf_s = sb.tile([128, KT * CO], F32)      # fp32 W' for the final iter
        jm_s = sb.tile([128, 128], F32)
        oc_s = sb.tile([128, 1], F32)
        vv_s = sb.tile([128, MB * NPAD], F32)    # squashed v (padded)
        s_s = sb.tile([128, MB * CO], F32)        # summed s-tilde
        b_s = sb.tile([128, KT * C], F32)         # b_ij expanded over i
        # 6 tail cols so the padded W'-scale read (c-count 16 per tile,
        # zero-multiplied) stays in bounds
        ct_s = sb.tile([128, KT * C + 6], F32)    # exp(b_ij)
        cs_s = sb.tile([128, C], F32)             # sum_t exp(b)
        p_s = sb.tile([128, KT * CO], F32)        # Wp .* G scratch
        mio_s = sb.tile([128, KT * C], F32)       # sum_o (Wp .* G)
        a2_s = sb.tile([128, MB * CO], F32)       # s^2
        t2_s = sb.tile([128, MB * CO], F32)       # s^2 + D^2
        r2_s = sb.tile([128, MB * CO], F32)       # 1/(s^2+D^2)
        ab_s = sb.tile([128, MB * CO], F32)       # |s|
        nn_s = sb.tile([128, MB * CO], F32)       # s*|s|
        dbc_s = sb.tile([128, C], F32)            # D broadcast over partitions
        dsq_s = sb.tile([128, C], F32)            # D^2
        ev_s = sb.tile([128, MB * CO], F32)       # PSUM->SBUF s-tilde evac
        dv_s = sb.tile([C, 1], F32)               # PSUM->SBUF D evac
        jnk_s = sb.tile([128, NPAD], F32)         # junk src for PE warmup

        # ---- DRAM bounce buffers for the collectives ----
        cc0_in = dram.tile([MB, 128, CO], F32)
        cc0_out = dram.tile([MB, 128, CO], F32, addr_space="Shared")
        CC1N = MB * 128 * CO + C
        cc1_in = dram.tile([CC1N], F32)
        cc1_out = dram.tile([CC1N], F32, addr_space="Shared")

        # ---- PE warmup: the HAM clock-gate starts cold (1.2GHz). ~3.5us of
        # junk matmuls during the input-load window flips it to 2.4GHz before
        # the first real matmul group (which otherwise runs its first ~3.4us
        # at half rate). Sized to end as the first load chunks land. ----
        nc.vector.memset(jnk_s[:], 0.0)
        if variant == "full":
            junk0 = psj.tile([128, NPAD], F32, name="junk", tag="junk")
            for _ in range(10):
                nc.tensor.matmul(
                    junk0[:], lhsT=jnk_s[:, 0:128], rhs=jnk_s[:],
                    start=True, stop=True,
                )

        # ---- loads (xt+wt first, in K-tile chunks: iteration-0 matmuls
        # start as soon as the first chunk of both lands) ----
        xt_ld = xt_s.rearrange("p (t b) -> p t b", t=KT)
        xt_src = xt_d.ap().rearrange("(t p) b -> p t b", p=128)
        wt_ld = wt_s.rearrange("p (t f) -> p t f", t=KT)
        wt_src = wt_d.ap().rearrange("(t p) f -> p t f", p=128)
        for lo, hi in ((0, 3), (3, 6), (6, KT)):
            nc.sync.dma_start(out=xt_ld[:, lo:hi], in_=xt_src[:, lo:hi])
            nc.sync.dma_start(out=wt_ld[:, lo:hi], in_=wt_src[:, lo:hi])
        nc.sync.dma_start(out=jm_s[:], in_=jm_d.ap())
        nc.sync.dma_start(out=oc_s[:], in_=oc_d.ap())
        nc.sync.dma_start(
            out=xn_s.rearrange("p (m k) -> p m k", m=MB),
            in_=xn_d.ap().rearrange("(m p) k -> p m k", p=128),
        )

        # vv pad columns [CO:NPAD) of each m-block: filled once with zeros
        # DMA'd from wt's (host-zeroed) pad region — Memset can't emit f32r.
        nc.vector.memset(b_s[:], 0.0)
        nc.vector.memset(ct_s[:, KT * C:], 0.0)

        xt3 = xt_s.rearrange("p (t b) -> p t b", t=KT)
        xn3 = xn_s.rearrange("p (m k) -> p m k", m=MB)
        wt3 = wt_s.rearrange("p (t f) -> p t f", t=KT)
        wp3 = wp_s.rearrange("p (t f) -> p t f", t=KT)
        vv3 = vv_s.rearrange("p (m f) -> p m f", m=MB)
        s3 = s_s.rearrange("p (m f) -> p m f", m=MB)
        ev3 = ev_s.rearrange("p (m f) -> p m f", m=MB)

        def stilde_matmul(w3, emit_block, f32_exact=False):
            """18 accumulating matmuls, two PSUM tiles (separate banks).

            After each m-block's 9 matmuls finish, the block is evacuated to
            ev_s and handed to emit_block(m) (the DMA toward the collective
            buffer / output) — so block 0's DMA latency hides behind block 1's
            matmuls instead of trailing the whole group.

            f32_exact: full-precision fp32 matmuls (final iteration — its
            result IS the output; routing-weight iterations tolerate f32r)."""
            for m in range(MB):
                sp = pss.tile([128, NPAD], F32, name=f"sps{m}", tag=f"sps{m}")
                for t in range(KT):
                    if f32_exact:
                        nc.tensor.matmul(
                            sp[:, 0:CO],
                            lhsT=_f32(xt3[:, t, ts(m, 128)]),
                            rhs=w3[:, t, :],
                            start=(t == 0),
                            stop=(t == KT - 1),
                        )
                    else:
                        nc.tensor.matmul(
                            sp[:],
                            lhsT=xt3[:, t, ts(m, 128)],
                            rhs=w3[:, t, :],
                            start=(t == 0),
                            stop=(t == KT - 1),
                        )
                nc.scalar.activation(ev3[:, m, :], sp[:, 0:CO], AF.Copy)
                emit_block(m)

        def squash(d_const=None):
            """vv <- squash(s / D): v = s*|s| / (D^2 + s^2), all on DVE.

            Per m-block so the first G matmul can start after block 0."""
            if d_const is None:
                nc.vector.tensor_mul(dsq_s[:], dbc_s[:], dbc_s[:])
            for m in range(MB):
                sf = s3[:, m, :]
                a2 = a2_s.rearrange("p (m f) -> p m f", m=MB)[:, m, :]
                ab = ab_s.rearrange("p (m f) -> p m f", m=MB)[:, m, :]
                t2 = t2_s.rearrange("p (m f) -> p m f", m=MB)[:, m, :]
                r2 = r2_s.rearrange("p (m f) -> p m f", m=MB)[:, m, :]
                nn = nn_s.rearrange("p (m f) -> p m f", m=MB)[:, m, :]
                # s^2 on ACT (idle here; walrus rejects scalar_tensor_tensor
                # on GPSIMD, so |s| stays on DVE) — shortens the serial DVE
                # chain that gates the G matmul block
                nc.scalar.activation(a2, sf, AF.Square)
                nc.vector.scalar_tensor_tensor(
                    ab, sf, -1.0, sf, op0=ALU.mult, op1=ALU.max)
                if d_const is not None:
                    nc.vector.tensor_scalar_add(t2, a2, float(d_const) ** 2)
                else:
                    nc.vector.tensor_add(
                        _free_bcast(t2_s, [[O, C], [1, O]],
                                    extra_offset=m * CO),
                        _free_bcast(a2_s, [[O, C], [1, O]],
                                    extra_offset=m * CO),
                        _free_bcast(dsq_s, [[1, C], [0, O]]),
                    )
                nc.vector.reciprocal(r2, t2)
                nc.vector.tensor_mul(nn, sf, ab)
                nc.vector.tensor_mul(vv3[:, m, 0:CO], nn, r2)

        def agreement(last=False):
            """G = X^T V; m = (1/B) sum_io Wp.*G; b += m; ct = exp(b); cs, D.

            last=True writes the scaled weights into the fp32 wpf_s tile for
            the exact final matmul instead of the f32r wp_s."""
            for t in range(KT):
                g_ps = psg.tile([128, NPAD], F32, name="gps", tag="gps")
                for m in range(MB):
                    nc.tensor.matmul(
                        g_ps[:, 0:CO],
                        lhsT=xn3[:, m, ts(t, 128)],
                        rhs=vv3[:, m, 0:CO],
                        start=(m == 0),
                        stop=(m == MB - 1),
                    )
                nc.vector.tensor_mul(
                    p_s[:, ts(t, CO)], _f32(wt3[:, t, 0:CO]), g_ps[:, 0:CO]
                )
                nc.vector.reduce_sum(
                    mio_s[:, ts(t, C)],
                    _free_bcast(p_s, [[16, C], [1, O]], extra_offset=t * CO),
                    axis=mybir.AxisListType.X,
                )
            me_ps = psx.tile([128, KT * C], F32, name="meps", tag="meps")
            nc.tensor.matmul(
                me_ps[:], lhsT=jm_s[:], rhs=mio_s[:], start=True, stop=True
            )
            nc.vector.tensor_add(b_s[:], b_s[:], me_ps[:])
            nc.scalar.activation(ct_s[:, 0:KT * C], b_s[:], AF.Exp)
            # W' = Wp * exp(b) per K-tile so the next matmul group pipelines
            # (broadcast exp(b) over o). The c-range runs to 16, not C=10,
            # covering the pad cols too: wt pads are 0, so wp pads = 0 with
            # no separate (f32r-incapable) memset.
            if last:
                for t in range(KT):
                    nc.vector.tensor_mul(
                        _free_bcast(wpf_s, [[16, C], [1, O]],
                                    extra_offset=t * CO),
                        _free_bcast(_f32(wt_s), [[16, C], [1, O]],
                                    extra_offset=t * NPAD),
                        _free_bcast(ct_s, [[1, C], [0, O]],
                                    extra_offset=t * C),
                    )
            else:
                cpt = NPAD // O
                for t in range(KT):
                    nc.vector.tensor_mul(
                        _free_bcast(wp_s, [[16, cpt], [1, O]],
                                    extra_offset=t * NPAD),
                        _free_bcast(_f32(wt_s), [[16, cpt], [1, O]],
                                    extra_offset=t * NPAD),
                        _free_bcast(ct_s, [[1, cpt], [0, O]],
                                    extra_offset=t * C),
                    )
            # D-tail after the W' loop: cs/D only ride the collective buffer
            # or output DMA, while W' tiles gate the next matmul group — so
            # let the DVE service W' first.
            nc.vector.reduce_sum(
                cs_s[:],
                _free_bcast(ct_s, [[1, C], [C, KT]]),
                axis=mybir.AxisListType.X,
            )
            d_ps = psx.tile([C, 1], F32, name="dps", tag="dps")
            nc.tensor.matmul(d_ps[:], lhsT=cs_s[:], rhs=oc_s[:], start=True, stop=True)
            nc.scalar.activation(dv_s[:], d_ps[:], AF.Copy)

        def allreduce(cc_in, cc_out):
            if variant == "nocc":
                nc.sync.dma_start(out=cc_out, in_=cc_in)
            else:
                nc.gpsimd.collective_compute(
                    "AllReduce",
                    mybir.AluOpType.add,
                    replica_groups=[list(range(NCORES))],
                    ins=[cc_in.opt()],
                    outs=[cc_out.opt()],
                )
                # PE stays idle ~17us through the AllReduce sandwich — past
                # the ~3.4us HAM window, so the post-sync matmul block would
                # restart at the cold 1.2GHz clock (~2x slower). Keep the HAM
                # warm with back-to-back junk matmuls sized to end ~3us
                # before the expected sync completion (PE tolerates <3.4us
                # of trailing idle without rethrottling). No data deps: they
                # fill the PE queue during the collective only.
                junk = psj.tile([128, NPAD], F32, name="junk", tag="junk")
                for _ in range(45):
                    nc.tensor.matmul(
                        junk[:], lhsT=xt3[:, 0, 0:128], rhs=wt3[:, 0, :],
                        start=True, stop=True,
                    )

        # ================= iteration 0 =================
        if variant == "mm_only":
            stilde_matmul(
                wt3, lambda m: nc.sync.dma_start(
                    out=outs_d.ap()[m], in_=ev3[:, m, :]))
            nc.sync.dma_start(out=outd_d.ap(), in_=ev_s[0:C, 0:1])
            return
        stilde_matmul(
            wt3, lambda m: nc.sync.dma_start(
                out=cc0_in[m], in_=ev3[:, m, :]))
        allreduce(cc0_in, cc0_out)
        for m in range(MB):
            nc.sync.dma_start(out=s3[:, m, :], in_=cc0_out[m])
        squash(d_const=float(R))           # D0 = R (softmax of zeros)
        agreement()                        # -> b1, exp(b1), D1 partial

        # ================= iteration 1 =================
        cc1s = cc1_in[0:MB * 128 * CO].rearrange("(m p f) -> m p f", m=MB, p=128)
        # D partial first: its data is ready before the matmul group, and
        # Tile's priority follows emission order
        nc.sync.dma_start(out=cc1_in[MB * 128 * CO:CC1N], in_=dv_s[:])
        stilde_matmul(
            wp3, lambda m: nc.sync.dma_start(
                out=cc1s[m], in_=ev3[:, m, :]))
        allreduce(cc1_in, cc1_out)
        cc1o = cc1_out[0:MB * 128 * CO].rearrange("(m p f) -> m p f", m=MB, p=128)
        # order: s-block0, D, s-block1 — HWDGE serializes ~0.6us per DMA, and
        # squash block 0 needs D (via dsq) by its third op
        nc.sync.dma_start(out=s3[:, 0, :], in_=cc1o[0])
        # D row from DRAM, replicated across all 128 partitions
        nc.sync.dma_start(
            out=dbc_s[:],
            in_=bass.AP(cc1_out.tensor, cc1_out.offset + MB * 128 * CO,
                        [[0, 128], [1, C]]),
        )
        nc.sync.dma_start(out=s3[:, 1, :], in_=cc1o[1])
        squash()
        agreement(last=True)               # -> b2, exp(b2), D2 partial (fp32 W')

        # ================= iteration 2 (exact fp32: result is the output) ==
        nc.sync.dma_start(out=outd_d.ap(), in_=dv_s[:])
        stilde_matmul(
            wpf_s.rearrange("p (t f) -> p t f", t=KT),
            lambda m: nc.sync.dma_start(out=outs_d.ap()[m], in_=ev3[:, m, :]),
            f32_exact=True)


def _prep_inputs(x, W):
    x = np.ascontiguousarray(np.asarray(x, np.float32))
    W = np.asarray(W, np.float32)
    Wp = np.ascontiguousarray(W.transpose(0, 3, 1, 2).reshape(R * I, C * O))
    Wpad = np.zeros((R * I, NPAD), np.float32)
    Wpad[:, :CO] = Wp
    jm = (np.kron(np.eye(16, dtype=np.float32), np.ones((8, 8), np.float32)) / B
          ).astype(np.float32)
    oc = np.full((128, 1), 1.0 / I, np.float32)
    in_maps = []
    for k in range(NCORES):
        xs = x[:, k * RL:(k + 1) * RL, :].reshape(B, KL)
        in_maps.append({
            "xt": np.ascontiguousarray(xs.T),
            "xn": np.ascontiguousarray(xs),
            "wt": np.ascontiguousarray(Wpad[k * KL:(k + 1) * KL]),
            "jm": jm,
            "oc": oc,
        })
    return in_maps


def _postprocess(results):
    s = np.zeros((MB, 128, CO), np.float64)
    D = np.zeros((C, 1), np.float64)
    for r in results:
        s += r["out_s"].astype(np.float64)
        D += r["out_d"].astype(np.float64)
    s = s.reshape(B, C, O)
    sn = s / D.reshape(C)[None, :, None]
    v = sn * np.abs(sn) / (1.0 + sn * sn)
    return v[..., None].astype(np.float32)


def _get_nc(variant="full"):
    key = f"nc_{variant}"
    if key not in _CACHE:
        _CACHE[key] = _build_nc(variant)
    return _CACHE[key]


def run_on_hw(x, W, **kw):
    """Run the bass kernel on the 8 cores; kw forwarded (e.g. trace=True)."""
    nc = _get_nc()
    in_maps = _prep_inputs(x, W)
    res = run_bass_kernel_spmd(nc, in_maps, core_ids=list(range(NCORES)), **kw)
    return _postprocess(res.results), res


def kernel(x, W):
    out, _ = run_on_hw(x, W)
    return out


# revision 55
# speedup vs baseline: 1.3359x; 1.0314x over previous
"""DigitCaps dynamic-routing kernel for 8 trn2 NeuronCores.

Math (reference):
    u_hat[b,r,c,o] = sum_i W[r,c,o,i] * x[b,r,i]        # never materialized!
    3 routing iters:
        c_ij = softmax(b_ij, axis=r)                     # (R, C)
        s    = einsum('rc,brco->bco', c_ij, u_hat)
        v    = squash(s)  elementwise: s|s|/(1+s^2)
        b_ij += mean_b <u_hat[b,r,c,:], v[b,c,:]>

Key restructuring:
  * s[b,(c,o)] = sum_{k=(r,i)} X[b,k] * (exp(b_ij[r,c]) * Wp[k,(c,o)]) / D[c]
    with Wp[(r,i),(c,o)] = W[r,c,o,i] and D[c] = sum_r exp(b_ij[r,c]).
    The softmax normalizer D commutes through the contraction, so each
    routing iteration needs exactly ONE AllReduce (partial s-tilde + partial D).
  * agreement: m[r,c] = (1/B) sum_{i,o} Wp[(r,i),(c,o)] * G[(r,i),(c,o)]
    with G = X^T V  (contraction over batch) -- a second dense matmul.
  * Routes are sharded 8-way (144 routes / core, K=1152 contraction rows);
    batch is replicated. The only cross-core traffic is the per-iteration
    AllReduce of (256x160 partial s + 10 partial D) = ~160KB.
  * The final iteration's AllReduce is folded into the host-side gather:
    each core emits its raw partial s-tilde and partial D; the host sums,
    normalizes and applies the squash.

Precision: plain fp32 matmuls (4 cyc/row on the PE). _USE_F32R=True switches
the routing-iteration matmuls to float32r (full-rate streaming with the
moving dim padded to 256, ~8us faster) but hardware f32r rounding costs
~1.2e-4 relative error through the routing-weight path vs 2.8e-6 at fp32,
which risks a scale-relative + fp32-envelope absmax gate — so it ships off.
"""

import numpy as np

import concourse.bass as bass
import concourse.mybir as mybir
from concourse import bacc, tile
from concourse.bass_utils import run_bass_kernel_spmd

B, R, C, O, I = 256, 1152, 10, 16, 8
NCORES = 8
RL = R // NCORES            # 144 routes per core
KL = RL * I                 # 1152 contraction rows per core
KT = KL // 128              # 9 K-tiles of 128
CO = C * O                  # 160
MB = B // 128               # 2 batch blocks of 128
NITER = 3

_USE_F32R = False
NPAD = 256 if _USE_F32R else CO   # moving-dim padding for full-rate f32r

F32 = mybir.dt.float32
F32R = mybir.dt.float32r
MMDT = F32R if _USE_F32R else F32
AF = mybir.ActivationFunctionType
ALU = mybir.AluOpType

_CACHE = {}


def _f32(ap):
    """View a (possibly float32r) AP as plain float32 for DVE/ACT ops."""
    return ap.bitcast(F32) if ap.dtype != F32 else ap


def _free_bcast(ap, dims, extra_offset=0):
    """Manual AP keeping the partition dim with custom free dims
    (steps in elements, step 0 = broadcast)."""
    return bass.AP(
        ap.tensor, ap.offset + extra_offset,
        [list(ap.ap[0])] + [list(d) for d in dims],
    )


def _build_nc(variant="full"):
    nc = bacc.Bacc(
        trn_type="TRN2",
        target_bir_lowering=False,
        debug=False,
        num_devices=NCORES,
    )

    xt_d = nc.dram_tensor("xt", [KL, B], MMDT, kind="ExternalInput")      # (r,i) x b
    xn_d = nc.dram_tensor("xn", [B, KL], F32, kind="ExternalInput")       # b x (r,i)
    wt_d = nc.dram_tensor("wt", [KL, NPAD], MMDT, kind="ExternalInput")   # padded Wp
    jm_d = nc.dram_tensor("jm", [128, 128], F32, kind="ExternalInput")    # kron(I16,1_8)/B
    oc_d = nc.dram_tensor("oc", [128, 1], F32, kind="ExternalInput")      # 1/I column
    outs_d = nc.dram_tensor("out_s", [MB, 128, CO], F32, kind="ExternalOutput")
    outd_d = nc.dram_tensor("out_d", [C, 1], F32, kind="ExternalOutput")

    with tile.TileContext(nc) as tc:
        _body(tc, xt_d, xn_d, wt_d, jm_d, oc_d, outs_d, outd_d, variant)
    nc.compile()
    return nc


def _body(tc, xt_d, xn_d, wt_d, jm_d, oc_d, outs_d, outd_d, variant="full"):
    nc = tc.nc
    ts = bass.ts

    with (
        tc.tile_pool(name="sb", bufs=1) as sb,
        tc.tile_pool(name="pss", bufs=1, space="PSUM") as pss,
        tc.tile_pool(name="psg", bufs=3, space="PSUM") as psg,
        tc.tile_pool(name="psx", bufs=1, space="PSUM") as psx,
        tc.tile_pool(name="psj", bufs=1, space="PSUM") as psj,
        tc.tile_pool(name="dram", bufs=1, space="DRAM") as dram,
    ):
        # ---- persistent SBUF tensors ----
        xt_s = sb.tile([128, KT * B], MMDT)       # lhsT for s-matmul
        xn_s = sb.tile([128, MB * KL], F32)      # lhsT for G-matmul
        wt_s = sb.tile([128, KT * NPAD], MMDT)    # Wp (padded)
        wp_s = sb.tile([128, KT * NPAD], MMDT)    # exp(b)-scaled Wp
        wpf_s = sb.tile([128, KT * CO], F32)      # fp32 W' for the final iter
        jm_s = sb.tile([128, 128], F32)
        oc_s = sb.tile([128, 1], F32)
        vv_s = sb.tile([128, MB * NPAD], F32)    # squashed v (padded)
        s_s = sb.tile([128, MB * CO], F32)        # summed s-tilde
        b_s = sb.tile([128, KT * C], F32)         # b_ij expanded over i
        # 6 tail cols so the padded W'-scale read (c-count 16 per tile,
        # zero-multiplied) stays in bounds
        ct_s = sb.tile([128, KT * C + 6], F32)    # exp(b_ij)
        cs_s = sb.tile([128, C], F32)             # sum_t exp(b)
        p_s = sb.tile([128, KT * CO], F32)        # Wp .* G scratch
        mio_s = sb.tile([128, KT * C], F32)       # sum_o (Wp .* G)
        a2_s = sb.tile([128, MB * CO], F32)       # s^2
        t2_s = sb.tile([128, MB * CO], F32)       # s^2 + D^2
        r2_s = sb.tile([128, MB * CO], F32)       # 1/(s^2+D^2)
        ab_s = sb.tile([128, MB * CO], F32)       # |s|
        nn_s = sb.tile([128, MB * CO], F32)       # s*|s|
        dbc_s = sb.tile([128, C], F32)            # D broadcast over partitions
        dsq_s = sb.tile([128, C], F32)            # D^2
        ev_s = sb.tile([128, MB * CO], F32)       # PSUM->SBUF s-tilde evac
        dv_s = sb.tile([C, 1], F32)               # PSUM->SBUF D evac
        jnk_s = sb.tile([128, NPAD], F32)         # junk src for PE warmup

        # ---- DRAM bounce buffers for the collectives ----
        cc0_in = dram.tile([MB, 128, CO], F32)
        cc0_out = dram.tile([MB, 128, CO], F32, addr_space="Shared")
        CC1N = MB * 128 * CO + C
        cc1_in = dram.tile([CC1N], F32)
        cc1_out = dram.tile([CC1N], F32, addr_space="Shared")

        # ---- PE warmup: the HAM clock-gate starts cold (1.2GHz). ~3.5us of
        # junk matmuls during the input-load window flips it to 2.4GHz before
        # the first real matmul group (which otherwise runs its first ~3.4us
        # at half rate). Sized to end as the first load chunks land. ----
        nc.vector.memset(jnk_s[:], 0.0)
        if variant == "full":
            junk0 = psj.tile([128, NPAD], F32, name="junk", tag="junk")
            for _ in range(10):
                nc.tensor.matmul(
                    junk0[:], lhsT=jnk_s[:, 0:128], rhs=jnk_s[:],
                    start=True, stop=True,
                )

        # ---- loads (xt+wt first, in K-tile chunks: iteration-0 matmuls
        # start as soon as the first chunk of both lands) ----
        xt_ld = xt_s.rearrange("p (t b) -> p t b", t=KT)
        xt_src = xt_d.ap().rearrange("(t p) b -> p t b", p=128)
        wt_ld = wt_s.rearrange("p (t f) -> p t f", t=KT)
        wt_src = wt_d.ap().rearrange("(t p) f -> p t f", p=128)
        # first chunk minimal (1 K-tile) so the first matmul starts ~1.2us
        # earlier; later chunks stream faster than the matmuls consume them
        for lo, hi in ((0, 1), (1, 4), (4, KT)):
            nc.sync.dma_start(out=xt_ld[:, lo:hi], in_=xt_src[:, lo:hi])
            nc.sync.dma_start(out=wt_ld[:, lo:hi], in_=wt_src[:, lo:hi])
        nc.sync.dma_start(out=jm_s[:], in_=jm_d.ap())
        nc.sync.dma_start(out=oc_s[:], in_=oc_d.ap())
        nc.sync.dma_start(
            out=xn_s.rearrange("p (m k) -> p m k", m=MB),
            in_=xn_d.ap().rearrange("(m p) k -> p m k", p=128),
        )

        # vv pad columns [CO:NPAD) of each m-block: filled once with zeros
        # DMA'd from wt's (host-zeroed) pad region — Memset can't emit f32r.
        nc.vector.memset(b_s[:], 0.0)
        nc.vector.memset(ct_s[:, KT * C:], 0.0)

        xt3 = xt_s.rearrange("p (t b) -> p t b", t=KT)
        xn3 = xn_s.rearrange("p (m k) -> p m k", m=MB)
        wt3 = wt_s.rearrange("p (t f) -> p t f", t=KT)
        wp3 = wp_s.rearrange("p (t f) -> p t f", t=KT)
        vv3 = vv_s.rearrange("p (m f) -> p m f", m=MB)
        s3 = s_s.rearrange("p (m f) -> p m f", m=MB)
        ev3 = ev_s.rearrange("p (m f) -> p m f", m=MB)

        def stilde_matmul(w3, emit_block, f32_exact=False):
            """18 accumulating matmuls, two PSUM tiles (separate banks).

            After each m-block's 9 matmuls finish, the block is evacuated to
            ev_s and handed to emit_block(m) (the DMA toward the collective
            buffer / output) — so block 0's DMA latency hides behind block 1's
            matmuls instead of trailing the whole group.

            f32_exact: full-precision fp32 matmuls (final iteration — its
            result IS the output; routing-weight iterations tolerate f32r)."""
            for m in range(MB):
                sp = pss.tile([128, NPAD], F32, name=f"sps{m}", tag=f"sps{m}")
                for t in range(KT):
                    if f32_exact:
                        nc.tensor.matmul(
                            sp[:, 0:CO],
                            lhsT=_f32(xt3[:, t, ts(m, 128)]),
                            rhs=w3[:, t, :],
                            start=(t == 0),
                            stop=(t == KT - 1),
                        )
                    else:
                        nc.tensor.matmul(
                            sp[:],
                            lhsT=xt3[:, t, ts(m, 128)],
                            rhs=w3[:, t, :],
                            start=(t == 0),
                            stop=(t == KT - 1),
                        )
                nc.scalar.activation(ev3[:, m, :], sp[:, 0:CO], AF.Copy)
                emit_block(m)

        def squash(d_const=None):
            """vv <- squash(s / D): v = s*|s| / (D^2 + s^2), all on DVE.

            Per m-block so the first G matmul can start after block 0."""
            if d_const is None:
                nc.vector.tensor_mul(dsq_s[:], dbc_s[:], dbc_s[:])
            for m in range(MB):
                sf = s3[:, m, :]
                a2 = a2_s.rearrange("p (m f) -> p m f", m=MB)[:, m, :]
                ab = ab_s.rearrange("p (m f) -> p m f", m=MB)[:, m, :]
                t2 = t2_s.rearrange("p (m f) -> p m f", m=MB)[:, m, :]
                r2 = r2_s.rearrange("p (m f) -> p m f", m=MB)[:, m, :]
                nn = nn_s.rearrange("p (m f) -> p m f", m=MB)[:, m, :]
                # s^2 on ACT (idle here; walrus rejects scalar_tensor_tensor
                # on GPSIMD, so |s| stays on DVE) — shortens the serial DVE
                # chain that gates the G matmul block
                nc.scalar.activation(a2, sf, AF.Square)
                nc.vector.scalar_tensor_tensor(
                    ab, sf, -1.0, sf, op0=ALU.mult, op1=ALU.max)
                if d_const is not None:
                    nc.vector.tensor_scalar_add(t2, a2, float(d_const) ** 2)
                else:
                    nc.vector.tensor_add(
                        _free_bcast(t2_s, [[O, C], [1, O]],
                                    extra_offset=m * CO),
                        _free_bcast(a2_s, [[O, C], [1, O]],
                                    extra_offset=m * CO),
                        _free_bcast(dsq_s, [[1, C], [0, O]]),
                    )
                nc.vector.reciprocal(r2, t2)
                nc.vector.tensor_mul(nn, sf, ab)
                nc.vector.tensor_mul(vv3[:, m, 0:CO], nn, r2)

        def agreement(last=False):
            """G = X^T V; m = (1/B) sum_io Wp.*G; b += m; ct = exp(b); cs, D.

            last=True writes the scaled weights into the fp32 wpf_s tile for
            the exact final matmul instead of the f32r wp_s."""
            for t in range(KT):
                g_ps = psg.tile([128, NPAD], F32, name="gps", tag="gps")
                for m in range(MB):
                    nc.tensor.matmul(
                        g_ps[:, 0:CO],
                        lhsT=xn3[:, m, ts(t, 128)],
                        rhs=vv3[:, m, 0:CO],
                        start=(m == 0),
                        stop=(m == MB - 1),
                    )
                nc.vector.tensor_mul(
                    p_s[:, ts(t, CO)], _f32(wt3[:, t, 0:CO]), g_ps[:, 0:CO]
                )
                nc.vector.reduce_sum(
                    mio_s[:, ts(t, C)],
                    _free_bcast(p_s, [[16, C], [1, O]], extra_offset=t * CO),
                    axis=mybir.AxisListType.X,
                )
            me_ps = psx.tile([128, KT * C], F32, name="meps", tag="meps")
            nc.tensor.matmul(
                me_ps[:], lhsT=jm_s[:], rhs=mio_s[:], start=True, stop=True
            )
            nc.vector.tensor_add(b_s[:], b_s[:], me_ps[:])
            nc.scalar.activation(ct_s[:, 0:KT * C], b_s[:], AF.Exp)
            # W' = Wp * exp(b) per K-tile so the next matmul group pipelines
            # (broadcast exp(b) over o). The c-range runs to 16, not C=10,
            # covering the pad cols too: wt pads are 0, so wp pads = 0 with
            # no separate (f32r-incapable) memset.
            if last:
                for t in range(KT):
                    nc.vector.tensor_mul(
                        _free_bcast(wpf_s, [[16, C], [1, O]],
                                    extra_offset=t * CO),
                        _free_bcast(_f32(wt_s), [[16, C], [1, O]],
                                    extra_offset=t * NPAD),
                        _free_bcast(ct_s, [[1, C], [0, O]],
                                    extra_offset=t * C),
                    )
            else:
                cpt = NPAD // O
                for t in range(KT):
                    nc.vector.tensor_mul(
                        _free_bcast(wp_s, [[16, cpt], [1, O]],
                                    extra_offset=t * NPAD),
                        _free_bcast(_f32(wt_s), [[16, cpt], [1, O]],
                                    extra_offset=t * NPAD),
                        _free_bcast(ct_s, [[1, cpt], [0, O]],
                                    extra_offset=t * C),
                    )
            # D-tail after the W' loop: cs/D only ride the collective buffer
            # or output DMA, while W' tiles gate the next matmul group — so
            # let the DVE service W' first.
            nc.vector.reduce_sum(
                cs_s[:],
                _free_bcast(ct_s, [[1, C], [C, KT]]),
                axis=mybir.AxisListType.X,
            )
            d_ps = psx.tile([C, 1], F32, name="dps", tag="dps")
            nc.tensor.matmul(d_ps[:], lhsT=cs_s[:], rhs=oc_s[:], start=True, stop=True)
            nc.scalar.activation(dv_s[:], d_ps[:], AF.Copy)

        def allreduce(cc_in, cc_out):
            if variant == "nocc":
                nc.sync.dma_start(out=cc_out, in_=cc_in)
            else:
                nc.gpsimd.collective_compute(
                    "AllReduce",
                    mybir.AluOpType.add,
                    replica_groups=[list(range(NCORES))],
                    ins=[cc_in.opt()],
                    outs=[cc_out.opt()],
                )
                # PE stays idle ~17us through the AllReduce sandwich — past
                # the ~3.4us HAM window, so the post-sync matmul block would
                # restart at the cold 1.2GHz clock (~2x slower). Keep the HAM
                # warm with back-to-back junk matmuls sized to end ~3us
                # before the expected sync completion (PE tolerates <3.4us
                # of trailing idle without rethrottling). No data deps: they
                # fill the PE queue during the collective only.
                junk = psj.tile([128, NPAD], F32, name="junk", tag="junk")
                for _ in range(45):
                    nc.tensor.matmul(
                        junk[:], lhsT=xt3[:, 0, 0:128], rhs=wt3[:, 0, :],
                        start=True, stop=True,
                    )

        # ================= iteration 0 =================
        if variant == "mm_only":
            stilde_matmul(
                wt3, lambda m: nc.sync.dma_start(
                    out=outs_d.ap()[m], in_=ev3[:, m, :]))
            nc.sync.dma_start(out=outd_d.ap(), in_=ev_s[0:C, 0:1])
            return
        stilde_matmul(
            wt3, lambda m: nc.sync.dma_start(
                out=cc0_in[m], in_=ev3[:, m, :]))
        allreduce(cc0_in, cc0_out)
        for m in range(MB):
            nc.sync.dma_start(out=s3[:, m, :], in_=cc0_out[m])
        squash(d_const=float(R))           # D0 = R (softmax of zeros)
        agreement()                        # -> b1, exp(b1), D1 partial

        # ================= iteration 1 =================
        cc1s = cc1_in[0:MB * 128 * CO].rearrange("(m p f) -> m p f", m=MB, p=128)
        # D partial first: its data is ready before the matmul group, and
        # Tile's priority follows emission order
        nc.sync.dma_start(out=cc1_in[MB * 128 * CO:CC1N], in_=dv_s[:])
        stilde_matmul(
            wp3, lambda m: nc.sync.dma_start(
                out=cc1s[m], in_=ev3[:, m, :]))
        allreduce(cc1_in, cc1_out)
        cc1o = cc1_out[0:MB * 128 * CO].rearrange("(m p f) -> m p f", m=MB, p=128)
        # order: s-block0, D, s-block1 — HWDGE serializes ~0.6us per DMA, and
        # squash block 0 needs D (via dsq) by its third op
        nc.sync.dma_start(out=s3[:, 0, :], in_=cc1o[0])
        # D row from DRAM, replicated across all 128 partitions
        nc.sync.dma_start(
            out=dbc_s[:],
            in_=bass.AP(cc1_out.tensor, cc1_out.offset + MB * 128 * CO,
                        [[0, 128], [1, C]]),
        )
        nc.sync.dma_start(out=s3[:, 1, :], in_=cc1o[1])
        squash()
        agreement(last=True)               # -> b2, exp(b2), D2 partial (fp32 W')

        # ================= iteration 2 (exact fp32: result is the output) ==
        nc.sync.dma_start(out=outd_d.ap(), in_=dv_s[:])
        stilde_matmul(
            wpf_s.rearrange("p (t f) -> p t f", t=KT),
            lambda m: nc.sync.dma_start(out=outs_d.ap()[m], in_=ev3[:, m, :]),
            f32_exact=True)


def _prep_inputs(x, W):
    x = np.ascontiguousarray(np.asarray(x, np.float32))
    W = np.asarray(W, np.float32)
    Wp = np.ascontiguousarray(W.transpose(0, 3, 1, 2).reshape(R * I, C * O))
    Wpad = np.zeros((R * I, NPAD), np.float32)
    Wpad[:, :CO] = Wp
    jm = (np.kron(np.eye(16, dtype=np.float32), np.ones((8, 8), np.float32)) / B
          ).astype(np.float32)
    oc = np.full((128, 1), 1.0 / I, np.float32)
    in_maps = []
    for k in range(NCORES):
        xs = x[:, k * RL:(k + 1) * RL, :].reshape(B, KL)
        in_maps.append({
            "xt": np.ascontiguousarray(xs.T),
            "xn": np.ascontiguousarray(xs),
            "wt": np.ascontiguousarray(Wpad[k * KL:(k + 1) * KL]),
            "jm": jm,
            "oc": oc,
        })
    return in_maps


def _postprocess(results):
    s = np.zeros((MB, 128, CO), np.float64)
    D = np.zeros((C, 1), np.float64)
    for r in results:
        s += r["out_s"].astype(np.float64)
        D += r["out_d"].astype(np.float64)
    s = s.reshape(B, C, O)
    sn = s / D.reshape(C)[None, :, None]
    v = sn * np.abs(sn) / (1.0 + sn * sn)
    return v[..., None].astype(np.float32)


def _get_nc(variant="full"):
    key = f"nc_{variant}"
    if key not in _CACHE:
        _CACHE[key] = _build_nc(variant)
    return _CACHE[key]


def run_on_hw(x, W, **kw):
    """Run the bass kernel on the 8 cores; kw forwarded (e.g. trace=True)."""
    nc = _get_nc()
    in_maps = _prep_inputs(x, W)
    res = run_bass_kernel_spmd(nc, in_maps, core_ids=list(range(NCORES)), **kw)
    return _postprocess(res.results), res


def kernel(x, W):
    out, _ = run_on_hw(x, W)
    return out
